# revision 4
# baseline (speedup 1.0000x reference)
"""Trainium2 Bass kernel for ConvPosMultiHeadAttn_Order.

Sharding: 8 cores = (batch b in 0..3) x (head-group hg in 0..1), 8 heads/core.

Per-core decomposition (all matmuls fp32r = full-rate PE with RNE-11 input
rounding, fp32 accumulate):
  - x^T resident in SBUF; transposed projections produce q/k1/k2 per head with
    HOST-side weight column tricks:
      * Q lhsT  = [Wq_h | Wq_h]      -> PSUM [q_h; q_h]
      * K lhsT  = [Wk2_h | Wk1_h]    -> PSUM [k2_h; k1_h]
    plus pe-table projections for the relative-position terms.
  - Speaker-select folded into an extended 256-dim score contraction:
      score^T[k,q] = [q*sq; q*(1-sq)] . [KA; KB] + [qp*sq; qp*(1-sq)] . [KPA; KPB]
    where KA = sk?k1:k2, KB = sk?k2:k1 (built by copy + copy_predicated from
    the interleaved PSUM), q-side masks applied during PSUM eviction (one DVE
    multiply by a host mask tile [sq-rows; (1-sq)-rows]).
  - Causal mask: lower-triangular k-chunk tiles only; diagonal tiles get one
    gpsimd affine_select (fill 0 where k > q) after an ACT exp.
  - Softmax denominators: ones-column (scaled by umask) appended to V in the
    PV lhsT -> row 64 of the PV PSUM holds the per-query sums. umask also
    scales V rows (exactly reproduces the reference key masking).
  - Normalize via reciprocal + PE outer-product broadcast, written shifted
    into the packed FC lhsT; final FC matmul + DMA out.
Host sums the two head-group partial outputs per batch.
"""
import sys

sys.path.insert(0, "/opt/trn_rl_repo")

import numpy as np

D = 1024
L = 1024
B = 4
DH = 64
NH = 8          # heads per core
NCORES = 8

_cached = {}


def _pe_table():
    num = 1201
    half = DH // 2
    freq = np.exp(np.arange(half, dtype=np.float32) * (-np.log(10000.0) / (half - 1)))
    pos_vals = np.arange(-num // 2, num // 2, dtype=np.float32)
    ang = pos_vals[:, None] * freq[None, :]
    table = np.concatenate([np.sin(ang), np.cos(ang)], axis=1).astype(np.float32)
    table[0] = 0.0
    idx = np.arange(-(L // 2), L // 2) + (num // 2 + 1)
    return table[idx]  # [L, DH] float32


def _build_program():
    import concourse.bass as bass
    import concourse.mybir as mybir
    import concourse.tile as tile
    from concourse import bacc

    f32 = mybir.dt.float32
    f32r = mybir.dt.float32r
    Exp = mybir.ActivationFunctionType.Exp

    nc = bacc.Bacc(None, target_bir_lowering=False, debug=False)

    XT = nc.declare_dram_parameter("XT", [D, L], f32r, isOutput=False)
    WQK = nc.declare_dram_parameter("WQK", [NH, 2, D, 128], f32r, isOutput=False)
    WPOS = nc.declare_dram_parameter("WPOS", [NH, 2, DH, 128], f32r, isOutput=False)
    WV = nc.declare_dram_parameter("WV", [D, 512], f32r, isOutput=False)
    WFC = nc.declare_dram_parameter("WFC", [512, D], f32r, isOutput=False)
    PET = nc.declare_dram_parameter("PET", [DH, L], f32r, isOutput=False)
    MCM = nc.declare_dram_parameter("MCM", [128, L], f32, isOutput=False)
    SKM = nc.declare_dram_parameter("SKM", [128, L], mybir.dt.uint8, isOutput=False)
    ONES1 = nc.declare_dram_parameter("ONES1", [1, 128], f32r, isOutput=False)
    UMASKT = nc.declare_dram_parameter("UMASKT", [128, 8], f32, isOutput=False)
    OCOLREP = nc.declare_dram_parameter("OCOLREP", [128, 64], f32r, isOutput=False)
    Y = nc.declare_dram_parameter("Y", [L, D], f32, isOutput=True)

    with tile.TileContext(nc) as tc:
        with tc.tile_pool(name="const", bufs=1) as const, \
             tc.tile_pool(name="wstream", bufs=2) as wstream, \
             tc.tile_pool(name="qk", bufs=2) as qkpool, \
             tc.tile_pool(name="exps", bufs=8) as exps, \
             tc.tile_pool(name="small", bufs=2) as small, \
             tc.tile_pool(name="ktmp", bufs=3) as ktmpp, \
             tc.tile_pool(name="yt", bufs=2) as ytp, \
             tc.tile_pool(name="proj_ps", bufs=3, space="PSUM") as proj_ps, \
             tc.tile_pool(name="score_ps", bufs=3, space="PSUM") as score_ps, \
             tc.tile_pool(name="pv_ps", bufs=2, space="PSUM") as pv_ps:

            # ---- resident constants ----
            xt = []
            for k in range(8):
                t = const.tile([128, L], f32r, tag=f"xt{k}")
                nc.sync.dma_start(t[:], XT[k * 128:(k + 1) * 128, :])
                xt.append(t)
            pet = const.tile([DH, L], f32r, tag="pet")
            nc.sync.dma_start(pet[:], PET[:])
            mcm = const.tile([128, L], f32, tag="mcm")
            nc.sync.dma_start(mcm[:], MCM[:])
            skm = const.tile([128, L], mybir.dt.uint8, tag="skm")
            nc.sync.dma_start(skm[:], SKM[:])
            ones1 = const.tile([1, 128], f32r, tag="ones1")
            nc.sync.dma_start(ones1[:], ONES1[:])
            umaskt = const.tile([128, 8], f32, tag="umaskt")
            nc.sync.dma_start(umaskt[:], UMASKT[:])
            ocolrep = const.tile([128, 64], f32r, tag="ocolrep")
            nc.sync.dma_start(ocolrep[:], OCOLREP[:])
            wqp, wkp = [], []
            for h in range(NH):
                t0 = const.tile([DH, 128], f32r, tag=f"wqp{h}")
                nc.sync.dma_start(t0[:], WPOS[h, 0])
                wqp.append(t0)
                t1 = const.tile([DH, 128], f32r, tag=f"wkp{h}")
                nc.sync.dma_start(t1[:], WPOS[h, 1])
                wkp.append(t1)
            wv = []
            for k in range(8):
                t = const.tile([128, 512], f32r, tag=f"wv{k}")
                nc.sync.dma_start(t[:], WV[k * 128:(k + 1) * 128, :])
                wv.append(t)
            vext = []
            for tcn in range(8):
                t = const.tile([128, NH * 65], f32r, tag=f"vext{tcn}")
                vext.append(t)
            outn = []
            for g in range(4):
                t = const.tile([128, L], f32r, tag=f"outn{g}")
                outn.append(t)

            # ---- V phase: token-major V + ones/umask columns ----
            for tcn in range(8):
                ocols = vext[tcn][:].rearrange("p (h c) -> p h c", c=65)[:, :, 64]
                nc.sync.dma_start(ocols, OCOLREP[:, tcn * 8:(tcn + 1) * 8])
                psv = proj_ps.tile([128, 512], f32, tag="proj")
                for k in range(8):
                    nc.tensor.matmul(psv[:], xt[k][:, tcn * 128:(tcn + 1) * 128],
                                     wv[k][:], start=(k == 0), stop=(k == 7))
                for h in range(NH):
                    nc.scalar.activation(
                        vext[tcn][:, h * 65:h * 65 + 64],
                        psv[:, h * 64:(h + 1) * 64],
                        mybir.ActivationFunctionType.Copy,
                        scale=umaskt[:, tcn:tcn + 1])

            # ---- per-head projections + attention ----
            for h in range(NH):
                wq_t = wstream.tile([128, D], f32r, tag="wq")
                nc.sync.dma_start(
                    wq_t[:].rearrange("p (k c) -> p k c", c=128),
                    WQK[h, 0].rearrange("(k p) c -> p k c", p=128))
                wk_t = wstream.tile([128, D], f32r, tag="wk")
                nc.sync.dma_start(
                    wk_t[:].rearrange("p (k c) -> p k c", c=128),
                    WQK[h, 1].rearrange("(k p) c -> p k c", p=128))

                qsd = qkpool.tile([128, L], f32r, tag="qsd")
                qpsd = qkpool.tile([128, L], f32r, tag="qpsd")
                k1t = qkpool.tile([128, L], f32r, tag="k1t")
                k2t = qkpool.tile([128, L], f32r, tag="k2t")

                for nt in range(2):
                    ntsl = bass.ts(nt, 512)
                    psq = proj_ps.tile([128, 512], f32, tag="proj")
                    for k in range(8):
                        nc.tensor.matmul(psq[:], wq_t[:, k * 128:(k + 1) * 128],
                                         xt[k][:, ntsl], start=(k == 0), stop=(k == 7))
                    nc.vector.tensor_mul(qsd[:, ntsl], psq[:], mcm[:, ntsl])

                    psqp = proj_ps.tile([128, 512], f32, tag="proj")
                    nc.tensor.matmul(psqp[:], wqp[h][:], pet[:, ntsl],
                                     start=True, stop=True)
                    nc.vector.tensor_mul(qpsd[:, ntsl], psqp[:], mcm[:, ntsl])

                    psk = proj_ps.tile([128, 512], f32, tag="proj")
                    for k in range(8):
                        nc.tensor.matmul(psk[:], wk_t[:, k * 128:(k + 1) * 128],
                                         xt[k][:, ntsl], start=(k == 0), stop=(k == 7))
                    kt1 = ktmpp.tile([128, 512], f32, tag="kt")
                    nc.scalar.copy(kt1[:], psk[:])
                    nc.vector.copy_predicated(kt1[0:64, :], skm[0:64, ntsl],
                                              psk[64:128, :])
                    nc.vector.copy_predicated(kt1[64:128, :], skm[64:128, ntsl],
                                              psk[0:64, :])
                    nc.vector.tensor_copy(k1t[:, ntsl], kt1[:])

                    pskp = proj_ps.tile([128, 512], f32, tag="proj")
                    nc.tensor.matmul(pskp[:], wkp[h][:], pet[:, ntsl],
                                     start=True, stop=True)
                    kt2 = ktmpp.tile([128, 512], f32, tag="kt")
                    nc.scalar.copy(kt2[:], pskp[:])
                    nc.vector.copy_predicated(kt2[0:64, :], skm[0:64, ntsl],
                                              pskp[64:128, :])
                    nc.vector.copy_predicated(kt2[64:128, :], skm[64:128, ntsl],
                                              pskp[0:64, :])
                    nc.vector.tensor_copy(k2t[:, ntsl], kt2[:])

                for qt in range(2):
                    qtsl = bass.ts(qt, 512)
                    jmax = 4 * (qt + 1)
                    pvps = pv_ps.tile([65, 512], f32, tag="pv")
                    for j in range(jmax):
                        sps = score_ps.tile([128, 512], f32, tag="s")
                        nc.tensor.matmul(sps[:], k1t[:, j * 128:(j + 1) * 128],
                                         qsd[:, qtsl], start=True, stop=False)
                        nc.tensor.matmul(sps[:], k2t[:, j * 128:(j + 1) * 128],
                                         qpsd[:, qtsl], start=False, stop=True)
                        et = exps.tile([128, 512], f32r, tag="e")
                        r = j * 128 - qt * 512
                        if r < 0:
                            nc.scalar.activation(et[:], sps[:], Exp)
                        else:
                            nc.scalar.activation(et[:, r:512], sps[:, r:512], Exp)
                            # keep where y - x - r >= 0 (q >= k), else 0
                            nc.gpsimd.affine_select(
                                out=et[:], in_=et[:],
                                compare_op=mybir.AluOpType.is_ge,
                                fill=0.0, base=-r,
                                pattern=[[1, 512]], channel_multiplier=-1)
                        nc.tensor.matmul(pvps[:], vext[j][:, h * 65:(h + 1) * 65],
                                         et[:], start=(j == 0), stop=(j == jmax - 1))
                    # normalize: row 64 of pvps holds softmax denominators
                    rc = small.tile([1, 512], f32, tag="rc")
                    nc.vector.reciprocal(rc[:], pvps[64:65, :])
                    rcr = small.tile([1, 512], f32r, tag="rcr")
                    nc.vector.tensor_copy(rcr[:], rc[:])
                    bps = score_ps.tile([64, 512], f32, tag="s")
                    nc.tensor.matmul(bps[:], ones1[:, 0:64], rcr[:],
                                     start=True, stop=True)
                    bsb = small.tile([64, 512], f32, tag="bsb")
                    nc.scalar.copy(bsb[:], bps[:])
                    g, row0 = h // 2, (h % 2) * 64
                    nc.vector.tensor_mul(outn[g][row0:row0 + 64, qtsl],
                                         pvps[0:64, :], bsb[:])

            # ---- FC ----
            wfc = []
            for kc, tg in enumerate(["qsd", "qpsd", "k1t", "k2t"]):
                t = qkpool.tile([128, L], f32r, tag=tg)
                nc.sync.dma_start(t[:], WFC[kc * 128:(kc + 1) * 128, :])
                wfc.append(t)
            for tcn in range(8):
                tsl = bass.ts(tcn, 128)
                for ct in range(2):
                    ctsl = bass.ts(ct, 512)
                    yps = score_ps.tile([128, 512], f32, tag="s")
                    for kc in range(4):
                        nc.tensor.matmul(yps[:], outn[kc][:, tsl],
                                         wfc[kc][:, ctsl],
                                         start=(kc == 0), stop=(kc == 3))
                    yt = ytp.tile([128, 512], f32, tag="y")
                    nc.scalar.copy(yt[:], yps[:])
                    nc.sync.dma_start(Y[tcn * 128:(tcn + 1) * 128, ct * 512:(ct + 1) * 512],
                                      yt[:])

    nc.compile()
    return nc


def _host_inputs(embed, umask, qmask, W_qkv, W_pos, W_fc):
    pe = _pe_table()
    pet = np.ascontiguousarray(pe.T)  # [DH, L]
    ones1 = np.ones((1, 128), np.float32)
    in_maps = []
    for core in range(NCORES):
        b, hg = core // 2, core % 2
        sq = qmask[b].astype(np.float32)          # [L] in {0,1}
        um = umask[b].astype(np.float32)          # [L]
        mcm = np.empty((128, L), np.float32)
        mcm[0:64] = sq[None, :]
        mcm[64:128] = (1.0 - sq)[None, :]
        skm = np.broadcast_to(qmask[b].astype(np.uint8)[None, :], (128, L)).copy()
        wqk = np.empty((NH, 2, D, 128), np.float32)
        wpos = np.empty((NH, 2, DH, 128), np.float32)
        for h in range(NH):
            gh = hg * NH + h
            qc = W_qkv[:, 0 * D + gh * DH: 0 * D + (gh + 1) * DH]
            k1c = W_qkv[:, 1 * D + gh * DH: 1 * D + (gh + 1) * DH]
            k2c = W_qkv[:, 2 * D + gh * DH: 2 * D + (gh + 1) * DH]
            wqk[h, 0] = np.concatenate([qc, qc], axis=1)
            wqk[h, 1] = np.concatenate([k2c, k1c], axis=1)
            qpc = W_pos[:, 0 * D + gh * DH: 0 * D + (gh + 1) * DH]
            kp1c = W_pos[:, 1 * D + gh * DH: 1 * D + (gh + 1) * DH]
            kp2c = W_pos[:, 2 * D + gh * DH: 2 * D + (gh + 1) * DH]
            wpos[h, 0] = np.concatenate([qpc, qpc], axis=1)
            wpos[h, 1] = np.concatenate([kp2c, kp1c], axis=1)
        umaskt = um.reshape(8, 128).T.copy()                     # [128, 8]
        ocolrep = np.repeat(umaskt, 8, axis=1).reshape(128, 8, 8)
        ocolrep = ocolrep.transpose(0, 1, 2).reshape(128, 64)
        # column tc*8+j must equal umask chunk tc:
        ocolrep = np.repeat(umaskt[:, :, None], 8, axis=2).reshape(128, 64)
        in_maps.append({
            "XT": np.ascontiguousarray(embed[b].T).astype(np.float32),
            "WQK": wqk,
            "WPOS": wpos,
            "WV": np.ascontiguousarray(
                W_qkv[:, 3 * D + hg * 512: 3 * D + (hg + 1) * 512]).astype(np.float32),
            "WFC": np.ascontiguousarray(W_fc[hg * 512:(hg + 1) * 512, :]).astype(np.float32),
            "PET": pet,
            "MCM": mcm,
            "SKM": skm,
            "ONES1": ones1,
            "UMASKT": umaskt,
            "OCOLREP": np.ascontiguousarray(ocolrep),
        })
    return in_maps


def kernel(embed, umask, qmask, W_qkv, W_pos, W_fc):
    from concourse.bass_utils import run_bass_kernel_spmd

    embed = np.asarray(embed, dtype=np.float32)
    umask = np.asarray(umask)
    qmask = np.asarray(qmask)
    W_qkv = np.asarray(W_qkv, dtype=np.float32)
    W_pos = np.asarray(W_pos, dtype=np.float32)
    W_fc = np.asarray(W_fc, dtype=np.float32)

    if "nc" not in _cached:
        _cached["nc"] = _build_program()
    nc = _cached["nc"]

    in_maps = _host_inputs(embed, umask, qmask, W_qkv, W_pos, W_fc)
    res = run_bass_kernel_spmd(nc, in_maps, list(range(NCORES))).results

    y = np.empty((B, L, D), np.float32)
    for b in range(B):
        y[b] = res[2 * b]["Y"] + res[2 * b + 1]["Y"]
    return y
